# revision 1
# baseline (speedup 1.0000x reference)
"""2-layer GAT (PyG GATConv semantics) on 8 Trainium2 NeuronCores via Bass/Tile.

Contract: kernel(**inputs) takes the FULL inputs of reference.setup_inputs()
and returns the FULL [16, 4096, 128] float32 output.

Strategy (dst-node sharding, one SPMD program):
- Core c owns dst nodes [c*N/8, (c+1)*N/8). Host bins edges (+self loops) by
  dst into 128-node blocks and pads each block's edge list to a uniform
  K chunks of 128 edges (K = global max, so the program is core-uniform).
- All per-core address differences are folded into per-core DATA:
  * The L1 node table t1 is ROTATED per core (local row r = node
    (c*NSH + r) mod N) so "own shard" is always rows [0, NSH).
  * The L2 table t2 (AllGathered) uses a slice-major physical layout; hosts
    pass gather indices as physical rows.
- Phase A (replicated): t1[n] = [h1 | a_src1 | a_dst1] f32, computed from
  xT @ [W1|Wsrc1|Wdst1] (a-values fused into the matmul as extra columns).
- Phase B/C (edge aggregation): per 128-edge chunk, ONE indirect DMA gathers
  table rows by src; routing matrix M[e,d] = (dstloc[e]==d) built on DVE;
  PSUM-accumulated matmul  acc += M^T @ [v_hi|v_lo|w_hi|w_lo]  where
  v = h[src]*w is computed in f32 then split into bf16 halves (f32-accurate
  on the bf16 PE path), w = exp(leakyrelu(a_src[src]+a_dst[dst])).
  a_dst[dst] per edge is selected on-chip: M^T (PE transpose) @ a-window.
  Softmax max-subtraction is skipped (logits are O(1) here; exact in f32).
- ELU's -1 is folded out: t2 stores h2' = (elu+1)@W2; since sum(alpha)=1 the
  colsum(W2) correction folds into the output bias and the L2 logit constant.
"""

import os
import sys

import numpy as np

if "/opt/trn_rl_repo" not in sys.path:
    sys.path.insert(0, "/opt/trn_rl_repo")

_KLVL = int(os.environ.get("KLVL", "9"))  # debug: chunk-loop feature level
_KEPI = int(os.environ.get("KEPI", "9"))  # debug: epilogue feature level

import concourse.bass as bass
import concourse.bacc as bacc
import concourse.mybir as mybir
import concourse.tile as tile

F32 = mybir.dt.float32
BF16 = mybir.dt.bfloat16
I32 = mybir.dt.int32
AOP = mybir.AluOpType
ACT = mybir.ActivationFunctionType

NEG_SLOPE = 0.2
NCORES = 8
BLK = 128

T1W = 80   # t1 slots: [0:64] h1 f32, [64:72] asrc1, [72:80] adst1
T2W = 130  # t2 slots: [0:128] h2' f32, [128] asrc2', [129] adst2'


class Cfg:
    def __init__(self, n_nodes, d_in, h1, c1, d2, k, nslice):
        self.N = n_nodes
        self.D = d_in
        self.H1 = h1
        self.C1 = c1
        self.D1 = h1 * c1
        self.D2 = d2
        self.K = k
        self.NSH = n_nodes // NCORES
        self.NBLK = self.NSH // BLK
        self.NSLICE = nslice
        assert self.NSH % BLK == 0 and self.NBLK % nslice == 0


# ---------------------------------------------------------------------------
# host-side edge schedule
# ---------------------------------------------------------------------------
def _edge_schedule(src, dst, n_nodes):
    """src/dst int64 arrays (self loops included). Per-core per-block padded
    edge slots; slot (p, j) = edge p*K + j of the block's list."""
    nsh = n_nodes // NCORES
    nblk = nsh // BLK
    order = np.argsort(dst, kind="stable")
    src = src[order]
    dst = dst[order]
    blk_of = dst // BLK
    nblk_g = n_nodes // BLK
    counts = np.bincount(blk_of, minlength=nblk_g)
    k = int((int(counts.max()) + 127) // 128)
    starts = np.zeros(nblk_g + 1, dtype=np.int64)
    np.cumsum(counts, out=starts[1:])

    slots = 128 * k
    esrc = np.zeros((NCORES, nblk, slots), dtype=np.int64)
    edloc = np.full((NCORES, nblk, slots), -1.0, dtype=np.float32)
    for g in range(nblk_g):
        c, b = divmod(g, nblk)
        s0, s1 = int(starts[g]), int(starts[g + 1])
        n = s1 - s0
        esrc[c, b, :n] = src[s0:s1]
        edloc[c, b, :n] = (dst[s0:s1] - g * BLK).astype(np.float32)
    return k, esrc.reshape(NCORES, nblk, 128, k), edloc.reshape(NCORES, nblk, 128, k)


def _t2_phys(cfg):
    """node id -> physical t2 row (slice-major: slice, rank, local)."""
    N, NSH, NSLICE = cfg.N, cfg.NSH, cfg.NSLICE
    sl = NSH // NSLICE
    node = np.arange(N, dtype=np.int64)
    r = node // NSH
    loc = node % NSH
    s = loc // sl
    return (s * (sl * NCORES) + r * sl + (loc % sl)).astype(np.int64)


# ---------------------------------------------------------------------------
# device program
# ---------------------------------------------------------------------------
def build_program(cfg, c2_const, phases="abgc"):
    N, D, H1, D1, D2, K = cfg.N, cfg.D, cfg.H1, cfg.D1, cfg.D2, cfg.K
    NSH, NBLK, NSLICE = cfg.NSH, cfg.NBLK, cfg.NSLICE

    nc = bacc.Bacc("TRN2", target_bir_lowering=False, debug=False, num_devices=NCORES)

    xt = nc.dram_tensor("xt", [D, N], F32, kind="ExternalInput")
    wpack1 = nc.dram_tensor("wpack1", [D, D1 + 2 * H1], F32, kind="ExternalInput")
    w2 = nc.dram_tensor("w2", [D1, D2], F32, kind="ExternalInput")
    attsrc2r = nc.dram_tensor("attsrc2r", [128, D2], F32, kind="ExternalInput")
    attdst2r = nc.dram_tensor("attdst2r", [128, D2], F32, kind="ExternalInput")
    b1r = nc.dram_tensor("b1r", [128, D1], F32, kind="ExternalInput")
    b2effr = nc.dram_tensor("b2effr", [128, D2], F32, kind="ExternalInput")
    iota = nc.dram_tensor("iota", [128, 128], F32, kind="ExternalInput")
    iotac = nc.dram_tensor("iotac", [128, 1], F32, kind="ExternalInput")
    esrc1 = nc.dram_tensor("esrc1", [NBLK, 128, K], I32, kind="ExternalInput")
    esrc2 = nc.dram_tensor("esrc2", [NBLK, 128, K], I32, kind="ExternalInput")
    edloc = nc.dram_tensor("edloc", [NBLK, 128, K], F32, kind="ExternalInput")
    out = nc.dram_tensor("out", [NSH, D2], F32, kind="ExternalOutput")

    dump = os.environ.get("KDUMP", "") in ("1", "2")
    ikind = "ExternalOutput" if os.environ.get("KDUMP", "") == "1" else "Internal"
    t1 = nc.dram_tensor("t1", [N, T1W], F32, kind=ikind)
    t2s = nc.dram_tensor("t2s", [NSH, T2W], F32, kind=ikind)
    t2 = nc.dram_tensor("t2", [N, T2W], F32, kind="Internal", addr_space="Shared")

    with tile.TileContext(nc) as tc:
        with tc.tile_pool(name="const", bufs=1) as cp:
            con = {}
            for name, hndl in [
                ("wpack1", wpack1), ("w2", w2), ("attsrc2r", attsrc2r),
                ("attdst2r", attdst2r), ("b1r", b1r), ("b2effr", b2effr),
                ("iota", iota), ("iotac", iotac),
            ]:
                t = cp.tile(list(hndl.shape), F32, tag=name)
                nc.sync.dma_start(out=t[:], in_=hndl[:])
                con[name] = t
            ident_bf = cp.tile([128, 128], BF16)
            nc.vector.tensor_tensor(
                out=ident_bf[:], in0=con["iotac"][:].to_broadcast([128, 128]),
                in1=con["iota"][:], op=AOP.is_equal,
            )
            ident_f = cp.tile([128, 128], F32)
            nc.vector.tensor_tensor(
                out=ident_f[:], in0=con["iotac"][:].to_broadcast([128, 128]),
                in1=con["iota"][:], op=AOP.is_equal,
            )
            con["ident_bf"] = ident_bf
            con["ident_f"] = ident_f

            if "a" in phases:
                _phase_a(nc, tc, cfg, xt, con["wpack1"], t1)
            if "b" in phases:
                _edge_phase(nc, tc, cfg, 1, t1, t2s, t2 if "g" in phases else None,
                            esrc1, edloc, con, 0.0, None)
            if dump and "g" in phases:
                t2dump = nc.dram_tensor("t2dump", [N, T2W], F32, kind="ExternalOutput")
                with tc.tile_pool(name="dmp", bufs=2) as pdm:
                    for i in range(N // 128):
                        dt_ = pdm.tile([128, T2W], F32, tag="d")
                        nc.sync.dma_start(out=dt_[:], in_=t2[i * 128 : (i + 1) * 128, :])
                        nc.sync.dma_start(out=t2dump[i * 128 : (i + 1) * 128, :], in_=dt_[:])
            if "c" in phases:
                _edge_phase(nc, tc, cfg, 2, t2, t2s, None, esrc2, edloc, con,
                            -c2_const, out)

    nc.compile()
    return nc


def _phase_a(nc, tc, cfg, xt, wpack1_t, t1):
    N, D, D1, H1 = cfg.N, cfg.D, cfg.D1, cfg.H1
    wcols = D1 + 2 * H1
    assert wcols == T1W
    ntile = N // 128
    group = 8 if ntile % 8 == 0 else 1
    with (
        tc.tile_pool(name="pa_in", bufs=3) as pin,
        tc.tile_pool(name="pa_ps", bufs=4, space="PSUM") as pps,
        tc.tile_pool(name="pa_st", bufs=3) as pst,
    ):
        for mt in range(ntile // group):
            xt_t = pin.tile([D, 128 * group], F32, tag="xt")
            nc.sync.dma_start(
                out=xt_t[:], in_=xt[:, mt * 128 * group : (mt + 1) * 128 * group]
            )
            stg = pst.tile([128, group * T1W], F32, tag="stg")
            for s in range(group):
                ps = pps.tile([128, wcols], F32, tag="ps")
                nc.tensor.matmul(
                    out=ps[:], lhsT=xt_t[:, s * 128 : (s + 1) * 128],
                    rhs=wpack1_t[:], start=True, stop=True,
                )
                nc.scalar.copy(out=stg[:, s * T1W : (s + 1) * T1W], in_=ps[:])
            dst_ap = bass.AP(
                t1[:].tensor,
                mt * 128 * group * T1W,
                [[T1W, 128], [128 * T1W, group], [1, T1W]],
            )
            nc.sync.dma_start(
                out=dst_ap, in_=stg[:].rearrange("p (s w) -> p s w", w=T1W)
            )


def _edge_phase(nc, tc, cfg, layer, table, t2s, t2, esrc, edloc, con, logit_c, out):
    """layer 1: table=t1, produces t2s + sliced AllGather into t2.
    layer 2: table=t2 (gathers) + t2s (own a_dst window), produces out."""
    K, NBLK, NSLICE = cfg.K, cfg.NBLK, cfg.NSLICE
    BPS = NBLK // NSLICE
    SLN = cfg.NSH // NSLICE
    if layer == 1:
        D, H, TW, acol = cfg.D1, cfg.H1, T1W, 64 + cfg.H1
        awin_src = table  # t1, rotated: own shard = rows [0, NSH)
    else:
        D, H, TW, acol = cfg.D2, 1, T2W, 129
        awin_src = t2s
    RC = 2 * D + 2 * H
    L = f"e{layer}"
    with (
        tc.tile_pool(name=L + "_ix", bufs=2) as pix,
        tc.tile_pool(name=L + "_g", bufs=3) as pg,
        tc.tile_pool(name=L + "_m", bufs=3) as pm,
        tc.tile_pool(name=L + "_r", bufs=3) as pr,
        tc.tile_pool(name=L + "_acc", bufs=2, space="PSUM") as pacc,
        tc.tile_pool(name=L + "_mtp", bufs=2, space="PSUM") as pmtp,
        tc.tile_pool(name=L + "_ad", bufs=2, space="PSUM") as pad,
        tc.tile_pool(name=L + "_ep", bufs=2) as pep,
        tc.tile_pool(name=L + "_epp", bufs=1, space="PSUM") as pepp,
    ):
        for b in range(NBLK):
            src_t = pix.tile([128, K], I32, tag="src")
            nc.sync.dma_start(out=src_t[:], in_=esrc[b])
            dloc_t = pix.tile([128, K], F32, tag="dloc")
            nc.sync.dma_start(out=dloc_t[:], in_=edloc[b])
            adw = pix.tile([128, H], F32, tag="adw")
            nc.sync.dma_start(
                out=adw[:], in_=awin_src[b * BLK : (b + 1) * BLK, acol : acol + H]
            )
            adw_hl = pix.tile([128, 2 * H], BF16, tag="adwhl")
            nc.vector.tensor_copy(out=adw_hl[:, 0:H], in_=adw[:])
            nc.vector.tensor_tensor(
                out=adw_hl[:, H : 2 * H], in0=adw[:], in1=adw_hl[:, 0:H],
                op=AOP.subtract,
            )

            acc = pacc.tile([128, RC], F32, tag="acc")
            for j in range(K):
                gat = pg.tile([128, TW], F32, tag="gat")
                nc.gpsimd.indirect_dma_start(
                    out=gat[:], out_offset=None, in_=table[:],
                    in_offset=bass.IndirectOffsetOnAxis(ap=src_t[:, j : j + 1], axis=0),
                )
                m_t = pm.tile([128, 128], BF16, tag="m")
                nc.vector.tensor_tensor(
                    out=m_t[:], in0=dloc_t[:, j : j + 1].to_broadcast([128, 128]),
                    in1=con["iota"][:], op=AOP.is_equal,
                )
                w_t = pr.tile([128, H], F32, tag="w")
                if _KLVL >= 1:
                    mtp = pmtp.tile([128, 128], BF16, tag="mtp")
                    nc.tensor.transpose(
                        out=mtp[:], in_=m_t[:], identity=con["ident_bf"][:]
                    )
                    mts = pm.tile([128, 128], BF16, tag="mts")
                    nc.vector.tensor_copy(out=mts[:], in_=mtp[:])
                    ad = pad.tile([128, 2 * H], F32, tag="ad")
                    nc.tensor.matmul(
                        out=ad[:], lhsT=mts[:], rhs=adw_hl[:], start=True, stop=True
                    )
                    ad2 = pr.tile([128, H], F32, tag="ad2")
                    nc.vector.scalar_tensor_tensor(
                        out=ad2[:], in0=ad[:, 0:H], scalar=float(logit_c),
                        in1=gat[:, D : D + H], op0=AOP.add, op1=AOP.add,
                    )
                    lg = pr.tile([128, H], F32, tag="lg")
                    nc.vector.tensor_add(out=lg[:], in0=ad2[:], in1=ad[:, H : 2 * H])
                    lr = pr.tile([128, H], F32, tag="lr")
                    nc.vector.scalar_tensor_tensor(
                        out=lr[:], in0=lg[:], scalar=NEG_SLOPE, in1=lg[:],
                        op0=AOP.mult, op1=AOP.max,
                    )
                    nc.scalar.activation(out=w_t[:], in_=lr[:], func=ACT.Exp)
                else:
                    nc.vector.memset(w_t[:], 1.0)
                rhs = pr.tile([128, RC], BF16, tag="rhs")
                if _KLVL >= 2:
                    nc.vector.tensor_copy(out=rhs[:, 2 * D : 2 * D + H], in_=w_t[:])
                    nc.vector.tensor_tensor(
                        out=rhs[:, 2 * D + H : RC], in0=w_t[:],
                        in1=rhs[:, 2 * D : 2 * D + H], op=AOP.subtract,
                    )
                    vf = pr.tile([128, D], F32, tag="vf")
                    if H > 1:
                        nc.vector.tensor_tensor(
                            out=vf[:].rearrange("p (h c) -> p h c", h=H),
                            in0=gat[:, 0:D].rearrange("p (h c) -> p h c", h=H),
                            in1=_bc_hc(w_t[:], H, D // H),
                            op=AOP.mult,
                        )
                    else:
                        nc.vector.tensor_tensor(
                            out=vf[:], in0=gat[:, 0:D],
                            in1=w_t[:].to_broadcast([128, D]), op=AOP.mult,
                        )
                    nc.scalar.copy(out=rhs[:, 0:D], in_=vf[:])
                    nc.vector.tensor_tensor(
                        out=rhs[:, D : 2 * D], in0=vf[:], in1=rhs[:, 0:D],
                        op=AOP.subtract,
                    )
                else:
                    nc.vector.memset(rhs[:], 1.0)
                nc.tensor.matmul(
                    out=acc[:], lhsT=m_t[:], rhs=rhs[:],
                    start=(j == 0), stop=(j == K - 1),
                )

            # ---------------- block epilogue ------------------------------
            accs = pep.tile([128, RC], F32, tag="accs")
            nc.scalar.copy(out=accs[:], in_=acc[:])
            vv = pep.tile([128, D], F32, tag="vv")
            nc.vector.tensor_add(out=vv[:], in0=accs[:, 0:D], in1=accs[:, D : 2 * D])
            s_t = pep.tile([128, H], F32, tag="s")
            nc.vector.tensor_add(
                out=s_t[:], in0=accs[:, 2 * D : 2 * D + H], in1=accs[:, 2 * D + H : RC]
            )
            sinv = pep.tile([128, H], F32, tag="sinv")
            nc.vector.reciprocal(out=sinv[:], in_=s_t[:])
            if layer == 2:
                o1 = pep.tile([128, D], F32, tag="o1")
                nc.scalar.activation(
                    out=o1[:], in_=vv[:], func=ACT.Copy, scale=sinv[:, 0:1]
                )
                o2 = pep.tile([128, D], F32, tag="o2")
                nc.vector.tensor_add(out=o2[:], in0=o1[:], in1=con["b2effr"][:, 0:D])
                nc.sync.dma_start(out=out[b * BLK : (b + 1) * BLK, :], in_=o2[:])
                continue

            if _KEPI < 1:
                stg0 = pep.tile([128, T2W], F32, tag="stg2")
                nc.vector.memset(stg0[:], 0.0)
                nc.vector.tensor_copy(out=stg0[:, 0:1], in_=accs[:, 0:1])
                nc.sync.dma_start(out=t2s[b * BLK : (b + 1) * BLK, :], in_=stg0[:])
                continue
            H1, C1, D2 = cfg.H1, cfg.C1, cfg.D2
            y = pep.tile([128, D], F32, tag="y")
            nc.vector.tensor_tensor(
                out=y[:].rearrange("p (h c) -> p h c", h=H1),
                in0=vv[:].rearrange("p (h c) -> p h c", h=H1),
                in1=_bc_hc(sinv[:], H1, C1),
                op=AOP.mult,
            )
            nc.vector.tensor_add(out=y[:], in0=y[:], in1=con["b1r"][:, 0:D])
            tmin = pep.tile([128, D], F32, tag="tmin")
            nc.vector.tensor_scalar_min(out=tmin[:], in0=y[:], scalar1=0.0)
            e_t = pep.tile([128, D], F32, tag="e")
            nc.scalar.activation(out=e_t[:], in_=tmin[:], func=ACT.Exp)
            helu = pep.tile([128, D], F32, tag="helu")
            nc.vector.scalar_tensor_tensor(
                out=helu[:], in0=y[:], scalar=0.0, in1=e_t[:],
                op0=AOP.max, op1=AOP.add,
            )
            if _KEPI < 2:
                stg0 = pep.tile([128, T2W], F32, tag="stg2")
                nc.vector.memset(stg0[:], 0.0)
                nc.vector.tensor_copy(out=stg0[:, 0:1], in_=helu[:, 0:1])
                nc.sync.dma_start(out=t2s[b * BLK : (b + 1) * BLK, :], in_=stg0[:])
                continue
            htp = pepp.tile([cfg.D1, 128], F32, tag="htp")
            nc.tensor.transpose(out=htp[:], in_=helu[:], identity=con["ident_f"][:])
            hts = pep.tile([cfg.D1, 128], F32, tag="hts")
            nc.vector.tensor_copy(out=hts[:], in_=htp[:])
            h2p = pepp.tile([128, D2], F32, tag="h2p")
            nc.tensor.matmul(
                out=h2p[:], lhsT=hts[:], rhs=con["w2"][:], start=True, stop=True
            )
            if _KEPI < 3:
                stg0 = pep.tile([128, T2W], F32, tag="stg2")
                nc.vector.memset(stg0[:], 0.0)
                nc.vector.tensor_copy(out=stg0[:, 0:1], in_=h2p[:, 0:1])
                nc.sync.dma_start(out=t2s[b * BLK : (b + 1) * BLK, :], in_=stg0[:])
                continue
            stg2 = pep.tile([128, T2W], F32, tag="stg2")
            scr = pep.tile([128, D2], F32, tag="scr")
            nc.vector.tensor_tensor(
                out=scr[:], in0=h2p[:], in1=con["attsrc2r"][:], op=AOP.mult
            )
            nc.vector.reduce_sum(
                out=stg2[:, D2 : D2 + 1], in_=scr[:], axis=mybir.AxisListType.X
            )
            nc.vector.tensor_tensor(
                out=scr[:], in0=h2p[:], in1=con["attdst2r"][:], op=AOP.mult
            )
            nc.vector.reduce_sum(
                out=stg2[:, D2 + 1 : D2 + 2], in_=scr[:], axis=mybir.AxisListType.X
            )
            nc.scalar.copy(out=stg2[:, 0:D2], in_=h2p[:])
            nc.sync.dma_start(out=t2s[b * BLK : (b + 1) * BLK, :], in_=stg2[:])

            if t2 is not None and (b + 1) % BPS == 0:
                s = (b + 1) // BPS - 1
                nc.gpsimd.collective_compute(
                    "AllGather",
                    AOP.bypass,
                    replica_groups=[list(range(NCORES))],
                    ins=[t2s[s * SLN : (s + 1) * SLN, :]],
                    outs=[t2[s * SLN * NCORES : (s + 1) * SLN * NCORES, :]],
                )


def _bc_hc(w_ap, h, c):
    """[128, h] -> stride-0 broadcast view [128, h, c]."""
    a = [list(p) for p in w_ap.ap]
    return bass.AP(w_ap.tensor, w_ap.offset, [a[0], a[1], [0, c]])


# ---------------------------------------------------------------------------
# host glue
# ---------------------------------------------------------------------------
def prepare(x, seq, edges, W1, att_src1, att_dst1, b1, W2, att_src2,
            att_dst2, b2, nslice=4):
    nb, ncn, d = x.shape
    N = nb * ncn
    H1, C1 = att_src1.shape
    D1 = H1 * C1
    D2 = W2.shape[1]

    xf = (np.asarray(x, np.float32).reshape(N, d)
          * np.asarray(seq, np.float32).reshape(N, 1))
    src = np.concatenate([np.asarray(edges[0], np.int64), np.arange(N, dtype=np.int64)])
    dst = np.concatenate([np.asarray(edges[1], np.int64), np.arange(N, dtype=np.int64)])
    k, esrc_g, edloc = _edge_schedule(src, dst, N)
    cfg = Cfg(N, d, H1, C1, D2, k, nslice)

    w1 = np.asarray(W1, np.float32)
    wsrc = np.einsum("khc,hc->kh", w1.reshape(d, H1, C1), np.asarray(att_src1, np.float32))
    wdst = np.einsum("khc,hc->kh", w1.reshape(d, H1, C1), np.asarray(att_dst1, np.float32))
    wpack1 = np.concatenate([w1, wsrc, wdst], axis=1).astype(np.float32)

    w2a = np.asarray(W2, np.float32)
    colsum = w2a.sum(axis=0)
    a2s = np.asarray(att_src2, np.float32).reshape(-1)
    a2d = np.asarray(att_dst2, np.float32).reshape(-1)
    c2_const = float(colsum @ a2s) + float(colsum @ a2d)
    b2eff = (np.asarray(b2, np.float32) - colsum).astype(np.float32)

    attsrc2r = np.tile(a2s[None, :], (128, 1)).astype(np.float32)
    attdst2r = np.tile(a2d[None, :], (128, 1)).astype(np.float32)
    b1r = np.tile(np.asarray(b1, np.float32)[None, :], (128, 1)).astype(np.float32)
    b2effr = np.tile(b2eff[None, :], (128, 1)).astype(np.float32)
    iota = np.tile(np.arange(128, dtype=np.float32)[None, :], (128, 1))
    iotac = np.arange(128, dtype=np.float32)[:, None].copy()

    phys = _t2_phys(cfg)
    in_maps = []
    for c in range(NCORES):
        rot = (np.arange(N, dtype=np.int64) + c * cfg.NSH) % N
        xt_c = np.ascontiguousarray(xf[rot].T)
        e1 = ((esrc_g[c] - c * cfg.NSH) % N).astype(np.int32)
        e2 = phys[esrc_g[c]].astype(np.int32)
        in_maps.append(
            {
                "xt": xt_c,
                "wpack1": wpack1,
                "w2": w2a,
                "attsrc2r": attsrc2r,
                "attdst2r": attdst2r,
                "b1r": b1r,
                "b2effr": b2effr,
                "iota": iota,
                "iotac": iotac,
                "esrc1": e1,
                "esrc2": e2,
                "edloc": edloc[c],
            }
        )
    return cfg, c2_const, in_maps


_CACHE = {}
LAST_RESULT = None


def kernel(**inputs) -> np.ndarray:
    from concourse.bass_utils import run_bass_kernel_spmd

    global LAST_RESULT
    x = np.asarray(inputs["x"])
    nb, ncn, d = x.shape
    cfg, c2_const, in_maps = prepare(**{k: inputs[k] for k in (
        "x", "seq", "edges", "W1", "att_src1", "att_dst1", "b1",
        "W2", "att_src2", "att_dst2", "b2")})

    key = (cfg.N, cfg.D, cfg.H1, cfg.C1, cfg.D2, cfg.K, cfg.NSLICE,
           round(c2_const, 10))
    if key not in _CACHE:
        _CACHE.clear()
        _CACHE[key] = build_program(cfg, c2_const)
    nc = _CACHE[key]

    res = run_bass_kernel_spmd(nc, in_maps, core_ids=list(range(NCORES)), trace=False)
    LAST_RESULT = res
    shards = [res.results[c]["out"] for c in range(NCORES)]
    full = np.concatenate(shards, axis=0)
    return full.reshape(nb, ncn, d).astype(np.float32)



# revision 12
# speedup vs baseline: 16.6758x; 16.6758x over previous
"""2-layer GAT (PyG GATConv semantics) on 8 Trainium2 NeuronCores via Bass/Tile.

Contract: kernel(**inputs) takes the FULL inputs of reference.setup_inputs()
and returns the FULL [16, 4096, 128] float32 output.

Strategy (dst-node sharding, one SPMD program, "one-dst-per-partition" grid):
- Core c owns dst nodes [c*N/8, (c+1)*N/8). Host packs each 128-dst block's
  edges into a [128 x T*KP] slot grid where each PARTITION row holds slots of
  exactly ONE dst (KP slots per row, ceil(deg/KP) rows per dst, T tiles of
  128 rows). This makes the routing matrix per tile a [128,128] one-hot of
  pdloc[p] (one is_equal per tile, not per 128-edge chunk), and a_dst
  selection one transpose+matmul per tile.
- Tables are bf16 with a FAKE row at index N whose a_src = -60000, so padded
  slots get w = exp(leakyrelu(-inf)) = 0 with no masks.
- Phase A (replicated): t1[n] = [h1 | a_src1 | a_dst1] bf16 from a single
  xT @ [W1|Wsrc1|Wdst1] bf16 matmul; t1 rows are ROTATED per core so own
  shard is rows [0, NSH).
- Edge phase (per block): ONE batched indirect gather of all T*KP*128 slot
  rows, ONE is_equal building all T routing matrices, batched logit/exp and
  value-weighting ops, and T*KP PSUM-accumulated matmuls
  acc[d, :] += M_t^T @ [v | w] where v = h[src]*w.
- Layer-2 linearity: out[d] = (sum alpha*g[src]) @ W2 with g = elu(y)+1, so
  the aggregation is 64-wide; W2 is applied once per dst block in the L2
  epilogue, a_src2 = g.(W2 a2s) folds the same way, and since sum(alpha)=1
  the elu/g shift folds into bias and logit constants.
- L2 table t2 (AllGathered, slice-major physical layout) rows are
  [g | a_src2' | a_dst2'] bf16 = 132B.
"""

import os
import sys

import numpy as np

if "/opt/trn_rl_repo" not in sys.path:
    sys.path.insert(0, "/opt/trn_rl_repo")

import concourse.bass as bass
import concourse.bacc as bacc
import concourse.mybir as mybir
import concourse.tile as tile

F32 = mybir.dt.float32
BF16 = mybir.dt.bfloat16
I32 = mybir.dt.int32
AOP = mybir.AluOpType
ACT = mybir.ActivationFunctionType

NEG_SLOPE = 0.2
NCORES = 8
BLK = 128
KP = int(os.environ.get("KKP", "8"))   # slots per partition-row
NEG_BIG = -60000.0

T1W = 80   # t1: [0:64] h1, [64:72] asrc1, [72:80] adst1 (gathers read 0:72)
T2W = 66   # t2: [0:64] g,  [64] asrc2',  [65] adst2'    (gathers read 0:65)


class Cfg:
    def __init__(self, n_nodes, d_in, h1, c1, d2, t_tiles, nslice):
        self.N = n_nodes
        self.D = d_in
        self.H1 = h1
        self.C1 = c1
        self.D1 = h1 * c1
        self.D2 = d2
        self.T = t_tiles
        self.TK = t_tiles * KP
        self.NSH = n_nodes // NCORES
        self.NBLK = self.NSH // BLK
        self.NSLICE = nslice
        assert self.NSH % BLK == 0 and self.NBLK % nslice == 0


# ---------------------------------------------------------------------------
# host-side edge schedule
# ---------------------------------------------------------------------------
def _edge_schedule(src, dst, n_nodes):
    """src/dst int64 (self loops included). One-dst-per-partition grid:
    esrc [NCORES, NBLK, 128, T*KP] (node id, FAKE=n_nodes for padding),
    pdloc [NCORES, NBLK, 128, T] (local dst in [0,128) or -1)."""
    nsh = n_nodes // NCORES
    nblk = nsh // BLK
    order = np.argsort(dst, kind="stable")
    src = src[order]
    dst = dst[order]
    ne = len(dst)

    deg = np.bincount(dst, minlength=n_nodes)
    starts = np.zeros(n_nodes + 1, dtype=np.int64)
    np.cumsum(deg, out=starts[1:])
    k_in_dst = np.arange(ne, dtype=np.int64) - starts[dst]

    rows_per_dst = (deg + KP - 1) // KP
    rowstart = np.zeros(n_nodes, dtype=np.int64)
    np.cumsum(rows_per_dst[:-1], out=rowstart[1:])
    first_dst_of_blk = (np.arange(n_nodes) // BLK) * BLK
    row_in_blk = rowstart - rowstart[first_dst_of_blk]

    rows_per_blk = (
        row_in_blk[BLK - 1 :: BLK] + rows_per_dst[BLK - 1 :: BLK]
    )
    t_tiles = int((int(rows_per_blk.max()) + 127) // 128)
    tk = t_tiles * KP

    r = row_in_blk[dst] + k_in_dst // KP          # row in [0, 128*T)
    tt = r // 128
    pp = r % 128
    jj = k_in_dst % KP
    col = tt * KP + jj
    g = dst // BLK
    cc = g // nblk
    bb = g % nblk

    esrc = np.full((NCORES, nblk, 128, tk), n_nodes, dtype=np.int64)
    esrc[cc, bb, pp, col] = src
    pdl = np.full((NCORES, nblk, 128, t_tiles), -1.0, dtype=np.float32)
    pdl[cc, bb, pp, tt] = (dst % BLK).astype(np.float32)
    return t_tiles, esrc, pdl


def _t2_phys(cfg):
    """node id -> physical t2 row (slice-major: slice, rank, local); fake->N."""
    N, NSH, NSLICE = cfg.N, cfg.NSH, cfg.NSLICE
    sl = NSH // NSLICE
    node = np.arange(N + 1, dtype=np.int64)
    r = node // NSH
    loc = node % NSH
    s = loc // sl
    phys = (s * (sl * NCORES) + r * sl + (loc % sl)).astype(np.int64)
    phys[N] = N
    return phys


# ---------------------------------------------------------------------------
# device program
# ---------------------------------------------------------------------------
def _ap(t, dims, offset=0):
    """SBUF/PSUM tile AP: keep partition dim, replace free dims.
    dims = [[stride, size], ...] in elements."""
    a = t[:]
    return bass.AP(a.tensor, a.offset + offset, [list(a.ap[0])] + [list(d) for d in dims])


def build_program(cfg, c2_const, phases="abgc"):
    N, D, H1, D1, D2 = cfg.N, cfg.D, cfg.H1, cfg.D1, cfg.D2
    NSH, NBLK, NSLICE, T, TK = cfg.NSH, cfg.NBLK, cfg.NSLICE, cfg.T, cfg.TK

    nc = bacc.Bacc("TRN2", target_bir_lowering=False, debug=False, num_devices=NCORES)

    xt = nc.dram_tensor("xt", [D, N], BF16, kind="ExternalInput")
    wpack1 = nc.dram_tensor("wpack1", [D, T1W], BF16, kind="ExternalInput")
    w2 = nc.dram_tensor("w2", [D1, D2], BF16, kind="ExternalInput")
    w2a2s = nc.dram_tensor("w2a2s", [128, D1], F32, kind="ExternalInput")
    w2a2d = nc.dram_tensor("w2a2d", [128, D1], F32, kind="ExternalInput")
    b1r = nc.dram_tensor("b1r", [128, D1], F32, kind="ExternalInput")
    b2effr = nc.dram_tensor("b2effr", [128, D2], F32, kind="ExternalInput")
    iota = nc.dram_tensor("iota", [128, 128], BF16, kind="ExternalInput")
    identd = nc.dram_tensor("identd", [128, 128], BF16, kind="ExternalInput")
    fr1 = nc.dram_tensor("fr1", [1, T1W], BF16, kind="ExternalInput")
    fr2 = nc.dram_tensor("fr2", [1, T2W], BF16, kind="ExternalInput")
    esrc1 = nc.dram_tensor("esrc1", [NBLK, 128, TK], I32, kind="ExternalInput")
    esrc2 = nc.dram_tensor("esrc2", [NBLK, 128, TK], I32, kind="ExternalInput")
    pdloc = nc.dram_tensor("pdloc", [NBLK, 128, T], BF16, kind="ExternalInput")
    out = nc.dram_tensor("out", [NSH, D2], F32, kind="ExternalOutput")

    t1 = nc.dram_tensor("t1", [N + 1, T1W], BF16, kind="Internal")
    t2s = nc.dram_tensor("t2s", [NSH, T2W], BF16, kind="Internal")
    t2 = nc.dram_tensor("t2", [N + 1, T2W], BF16, kind="Internal", addr_space="Shared")

    with tile.TileContext(nc) as tc:
        with tc.tile_pool(name="const", bufs=1) as cp:
            con = {}
            for name, hndl, dt_ in [
                ("wpack1", wpack1, BF16), ("w2", w2, BF16),
                ("w2a2s", w2a2s, F32), ("w2a2d", w2a2d, F32),
                ("b1r", b1r, F32), ("b2effr", b2effr, F32),
                ("iota", iota, BF16), ("ident", identd, BF16),
                ("fr1", fr1, BF16), ("fr2", fr2, BF16),
            ]:
                t = cp.tile(list(hndl.shape), dt_, tag=name)
                nc.sync.dma_start(out=t[:], in_=hndl[:])
                con[name] = t

            if "a" in phases:
                _phase_a(nc, tc, cfg, xt, con, t1)
            nc.sync.dma_start(out=t1[N : N + 1, :], in_=con["fr1"][:])
            nc.sync.dma_start(out=t2[N : N + 1, :], in_=con["fr2"][:])
            if "b" in phases:
                _edge_phase(nc, tc, cfg, 1, t1, t2s, t2 if "g" in phases else None,
                            esrc1, pdloc, con, 0.0, None)
            if "c" in phases:
                _edge_phase(nc, tc, cfg, 2, t2, t2s, None, esrc2, pdloc, con,
                            -c2_const, out)

    nc.compile()
    return nc


def _phase_a(nc, tc, cfg, xt, con, t1):
    N = cfg.N
    ntile = N // 128
    group = 8 if ntile % 8 == 0 else 1
    with (
        tc.tile_pool(name="pa_in", bufs=3) as pin,
        tc.tile_pool(name="pa_ps", bufs=4, space="PSUM") as pps,
        tc.tile_pool(name="pa_st", bufs=3) as pst,
    ):
        for mt in range(ntile // group):
            xt_t = pin.tile([cfg.D, 128 * group], BF16, tag="xt")
            nc.sync.dma_start(
                out=xt_t[:], in_=xt[:, mt * 128 * group : (mt + 1) * 128 * group]
            )
            stg = pst.tile([128, group * T1W], BF16, tag="stg")
            for s in range(group):
                ps = pps.tile([128, T1W], F32, tag="ps")
                nc.tensor.matmul(
                    out=ps[:], lhsT=xt_t[:, s * 128 : (s + 1) * 128],
                    rhs=con["wpack1"][:], start=True, stop=True,
                )
                nc.scalar.copy(out=stg[:, s * T1W : (s + 1) * T1W], in_=ps[:])
            dst_ap = bass.AP(
                t1[:].tensor,
                mt * 128 * group * T1W,
                [[T1W, 128], [128 * T1W, group], [1, T1W]],
            )
            nc.sync.dma_start(
                out=dst_ap, in_=stg[:].rearrange("p (s w) -> p s w", w=T1W)
            )


def _edge_phase(nc, tc, cfg, layer, table, t2s, t2, esrc, pdloc, con, logit_c, out):
    """layer 1: table=t1 (rotated, own shard = rows [0,NSH)), writes t2s and
    (sliced) AllGathers into t2.  layer 2: table=t2, writes out."""
    NBLK, NSLICE, T, TK = cfg.NBLK, cfg.NSLICE, cfg.T, cfg.TK
    BPS = NBLK // NSLICE
    SLN = cfg.NSH // NSLICE
    H1, C1, D1, D2 = cfg.H1, cfg.C1, cfg.D1, cfg.D2
    if layer == 1:
        D, H, TW, acol = D1, cfg.H1, T1W, 72
        awin_src = table
    else:
        D, H, TW, acol = D1, 1, T2W, 65
        awin_src = t2s
    GW = D + H          # gathered row prefix width (values + a_src)
    RC = D + H          # rhs/acc width: [v | w]
    L = f"e{layer}"
    with (
        tc.tile_pool(name=L + "_ix", bufs=2) as pix,
        tc.tile_pool(name=L + "_g", bufs=3) as pg,
        tc.tile_pool(name=L + "_m", bufs=2) as pm,
        tc.tile_pool(name=L + "_r", bufs=2) as pr,
        tc.tile_pool(name=L + "_acc", bufs=2, space="PSUM") as pacc,
        tc.tile_pool(name=L + "_acs", bufs=2, space="PSUM") as pacs,
        tc.tile_pool(name=L + "_mtp", bufs=1, space="PSUM") as pmtp,
        tc.tile_pool(name=L + "_ad", bufs=1, space="PSUM") as pad,
        tc.tile_pool(name=L + "_ep", bufs=2) as pep,
        tc.tile_pool(name=L + "_epp", bufs=1, space="PSUM") as pepp,
    ):
        for b in range(NBLK):
            src_t = pix.tile([128, TK], I32, tag="src")
            nc.sync.dma_start(out=src_t[:], in_=esrc[b])
            pdl_t = pix.tile([128, T], BF16, tag="pdl")
            nc.sync.dma_start(out=pdl_t[:], in_=pdloc[b])
            adw = pix.tile([128, H], BF16, tag="adw")
            nc.sync.dma_start(
                out=adw[:], in_=awin_src[b * BLK : (b + 1) * BLK, acol : acol + H]
            )

            # gather slot rows: [128, TK, GW] <- table[src]; one indirect DMA
            # per slot column ([128,1] offsets is the only HW-proven shape)
            gat = pg.tile([128, TK * GW], BF16, tag="gat")
            for c in range(TK):
                nc.gpsimd.indirect_dma_start(
                    out=_ap(gat, [[1, GW]], offset=c * GW),
                    out_offset=None, in_=table[:],
                    in_offset=bass.IndirectOffsetOnAxis(
                        ap=src_t[:, c : c + 1], axis=0),
                )

            # all T routing matrices in one is_equal
            m_all = pm.tile([128, T * 128], BF16, tag="m")
            nc.vector.tensor_tensor(
                out=_ap(m_all, [[128, T], [1, 128]]),
                in0=_ap(pdl_t, [[1, T], [0, 128]]),
                in1=_ap(con["iota"], [[0, T], [1, 128]]),
                op=AOP.is_equal,
            )

            # a_dst per partition-row, replicated over KP slots:
            # ad_all[p, (t,j,h)] = adw[pd_t[p], h] via M^T selection
            adw_rep = pix.tile([128, KP * H], BF16, tag="adwr")
            nc.vector.tensor_copy(
                out=_ap(adw_rep, [[H, KP], [1, H]]),
                in_=_ap(adw, [[0, KP], [1, H]]),
            )
            ad_all = pad.tile([128, TK * H], F32, tag="ad")
            for t in range(T):
                mtp = pmtp.tile([128, 128], BF16, tag="mtp")
                nc.tensor.transpose(
                    out=mtp[:], in_=m_all[:, t * 128 : (t + 1) * 128],
                    identity=con["ident"][:],
                )
                mts = pm.tile([128, 128], BF16, tag="mts")
                nc.vector.tensor_copy(out=mts[:], in_=mtp[:])
                nc.tensor.matmul(
                    out=ad_all[:, t * KP * H : (t + 1) * KP * H], lhsT=mts[:],
                    rhs=adw_rep[:], start=True, stop=True,
                )

            # logits -> w (batched over the whole block; all APs <= 3D)
            lg = pr.tile([128, TK * H], F32, tag="lg")
            nc.vector.scalar_tensor_tensor(
                out=lg[:],
                in0=ad_all[:],
                scalar=float(logit_c),
                in1=_ap(gat, [[GW, TK], [1, H]], offset=D),
                op0=AOP.add, op1=AOP.add,
            )
            lr = pr.tile([128, TK * H], F32, tag="lr")
            nc.vector.scalar_tensor_tensor(
                out=lr[:], in0=lg[:], scalar=NEG_SLOPE, in1=lg[:],
                op0=AOP.mult, op1=AOP.max,
            )
            # w expanded per value column: w_exp[p, (slot, h, c)] = w[p, slot, h]
            CC = D // H
            w_exp = pr.tile([128, TK * D], BF16, tag="wx")
            nc.scalar.activation(
                out=_ap(w_exp, [[CC, TK * H], [1, CC]]),
                in_=_ap(lr, [[1, TK * H], [0, CC]]),
                func=ACT.Exp,
            )
            v_t = pr.tile([128, TK * D], BF16, tag="v")
            nc.vector.tensor_tensor(
                out=v_t[:],
                in0=_ap(gat, [[GW, TK], [1, D]]),
                in1=w_exp[:],
                op=AOP.mult,
            )

            # routed accumulation: acc_v += M^T v, acc_s += M^T w
            acc_v = pacc.tile([128, D], F32, tag="accv")
            acc_s = pacs.tile([128, H], F32, tag="accs")
            for t in range(T):
                lhsT = m_all[:, t * 128 : (t + 1) * 128]
                for j in range(KP):
                    c = t * KP + j
                    first = c == 0
                    last = c == TK - 1
                    nc.tensor.matmul(
                        out=acc_v[:], lhsT=lhsT,
                        rhs=_ap(v_t, [[1, D]], offset=c * D),
                        start=first, stop=last,
                    )
                    nc.tensor.matmul(
                        out=acc_s[:], lhsT=lhsT,
                        rhs=_ap(w_exp, [[CC, H]], offset=c * D),
                        start=first, stop=last,
                    )

            # ---------------- block epilogue ------------------------------
            sinv = pep.tile([128, H], F32, tag="sinv")
            nc.vector.reciprocal(out=sinv[:], in_=acc_s[:])

            if layer == 2:
                o1 = pep.tile([128, D1], F32, tag="o1")
                nc.scalar.activation(
                    out=o1[:], in_=acc_v[:], func=ACT.Copy, scale=sinv[:, 0:1]
                )
                o1b = pep.tile([128, D1], BF16, tag="o1b")
                nc.vector.tensor_copy(out=o1b[:], in_=o1[:])
                atp = pepp.tile([D1, 128], BF16, tag="atp")
                nc.tensor.transpose(out=atp[:], in_=o1b[:], identity=con["ident"][:])
                ats = pep.tile([D1, 128], BF16, tag="ats")
                nc.vector.tensor_copy(out=ats[:], in_=atp[:])
                ops = pepp.tile([128, D2], F32, tag="ops")
                nc.tensor.matmul(
                    out=ops[:], lhsT=ats[:], rhs=con["w2"][:], start=True, stop=True
                )
                o2 = pep.tile([128, D2], F32, tag="o2")
                nc.vector.tensor_add(out=o2[:], in0=ops[:], in1=con["b2effr"][:])
                nc.sync.dma_start(out=out[b * BLK : (b + 1) * BLK, :], in_=o2[:])
                continue

            # layer 1: y = acc_v/s + b1; t2s row = [elu(y)|asrc'|adst']
            y = pep.tile([128, D1], F32, tag="y")
            nc.vector.tensor_tensor(
                out=_ap(y, [[C1, H1], [1, C1]]),
                in0=_ap(acc_v, [[C1, H1], [1, C1]]),
                in1=_ap(sinv, [[1, H1], [0, C1]]),
                op=AOP.mult,
            )
            nc.vector.tensor_add(out=y[:], in0=y[:], in1=con["b1r"][:])
            tmin = pep.tile([128, D1], F32, tag="tmin")
            nc.vector.tensor_scalar_min(out=tmin[:], in0=y[:], scalar1=0.0)
            e_t = pep.tile([128, D1], F32, tag="e")
            nc.scalar.activation(out=e_t[:], in_=tmin[:], func=ACT.Exp)
            g_t = pep.tile([128, D1], F32, tag="g")
            nc.vector.scalar_tensor_tensor(
                out=g_t[:], in0=y[:], scalar=0.0, in1=e_t[:],
                op0=AOP.max, op1=AOP.add,
            )
            # center: store elu = g - 1 (avoids bf16 cancellation in A@W2)
            eluf = pep.tile([128, D1], F32, tag="eluf")
            nc.vector.tensor_scalar_add(out=eluf[:], in0=g_t[:], scalar1=-1.0)
            stg2 = pep.tile([128, T2W], BF16, tag="stg2")
            nc.vector.tensor_copy(out=stg2[:, 0:D1], in_=eluf[:])
            scr = pep.tile([128, D1], F32, tag="scr")
            nc.vector.tensor_tensor(
                out=scr[:], in0=eluf[:], in1=con["w2a2s"][:], op=AOP.mult
            )
            with nc.allow_low_precision(reason="bf16 logit terms, tol 2e-2"):
                nc.vector.reduce_sum(
                    out=stg2[:, D1 : D1 + 1], in_=scr[:], axis=mybir.AxisListType.X
                )
            nc.vector.tensor_tensor(
                out=scr[:], in0=eluf[:], in1=con["w2a2d"][:], op=AOP.mult
            )
            with nc.allow_low_precision(reason="bf16 logit terms, tol 2e-2"):
                nc.vector.reduce_sum(
                    out=stg2[:, D1 + 1 : D1 + 2], in_=scr[:], axis=mybir.AxisListType.X
                )
            nc.sync.dma_start(out=t2s[b * BLK : (b + 1) * BLK, :], in_=stg2[:])

            if t2 is not None and (b + 1) % BPS == 0:
                s = (b + 1) // BPS - 1
                nc.gpsimd.collective_compute(
                    "AllGather",
                    AOP.bypass,
                    replica_groups=[list(range(NCORES))],
                    ins=[t2s[s * SLN : (s + 1) * SLN, :]],
                    outs=[t2[s * SLN * NCORES : (s + 1) * SLN * NCORES, :]],
                )


# ---------------------------------------------------------------------------
# host glue
# ---------------------------------------------------------------------------
def prepare(x, seq, edges, W1, att_src1, att_dst1, b1, W2, att_src2,
            att_dst2, b2, nslice=4):
    nb, ncn, d = x.shape
    N = nb * ncn
    H1, C1 = att_src1.shape
    D1 = H1 * C1
    D2 = W2.shape[1]

    xf = (np.asarray(x, np.float32).reshape(N, d)
          * np.asarray(seq, np.float32).reshape(N, 1))
    src = np.concatenate([np.asarray(edges[0], np.int64), np.arange(N, dtype=np.int64)])
    dst = np.concatenate([np.asarray(edges[1], np.int64), np.arange(N, dtype=np.int64)])
    t_tiles, esrc_g, pdl = _edge_schedule(src, dst, N)
    cfg = Cfg(N, d, H1, C1, D2, t_tiles, nslice)

    w1 = np.asarray(W1, np.float32)
    wsrc = np.einsum("khc,hc->kh", w1.reshape(d, H1, C1), np.asarray(att_src1, np.float32))
    wdst = np.einsum("khc,hc->kh", w1.reshape(d, H1, C1), np.asarray(att_dst1, np.float32))
    wpack1 = np.concatenate([w1, wsrc, wdst], axis=1).astype(np.float32)

    w2a = np.asarray(W2, np.float32)
    a2s = np.asarray(att_src2, np.float32).reshape(-1)
    a2d = np.asarray(att_dst2, np.float32).reshape(-1)
    # t2 stores centered elu values, so no colsum / logit-constant folds
    c2_const = 0.0
    b2eff = np.asarray(b2, np.float32)
    w2s = w2a @ a2s    # [D1]
    w2d = w2a @ a2d

    def bf(a):
        import jax.numpy as jnp
        return np.asarray(jnp.asarray(a, jnp.bfloat16))

    w2a2s = np.tile(w2s[None, :], (128, 1)).astype(np.float32)
    w2a2d = np.tile(w2d[None, :], (128, 1)).astype(np.float32)
    b1r = np.tile(np.asarray(b1, np.float32)[None, :], (128, 1)).astype(np.float32)
    b2effr = np.tile(b2eff[None, :], (128, 1)).astype(np.float32)
    iota = np.tile(np.arange(128, dtype=np.float32)[None, :], (128, 1))
    ident = np.eye(128, dtype=np.float32)
    fr1 = np.zeros((1, T1W), np.float32)
    fr1[0, D1 : D1 + H1] = NEG_BIG
    fr2 = np.zeros((1, T2W), np.float32)
    fr2[0, D1] = NEG_BIG

    phys = _t2_phys(cfg)
    in_maps = []
    for c in range(NCORES):
        rot = (np.arange(N, dtype=np.int64) + c * cfg.NSH) % N
        xt_c = bf(np.ascontiguousarray(xf[rot].T))
        e1 = np.where(
            esrc_g[c] == N, N, (esrc_g[c] - c * cfg.NSH) % N
        ).astype(np.int32)
        e2 = phys[esrc_g[c]].astype(np.int32)
        in_maps.append(
            {
                "xt": xt_c,
                "wpack1": bf(wpack1),
                "w2": bf(w2a),
                "w2a2s": w2a2s,
                "w2a2d": w2a2d,
                "b1r": b1r,
                "b2effr": b2effr,
                "iota": bf(iota),
                "identd": bf(ident),
                "fr1": bf(fr1),
                "fr2": bf(fr2),
                "esrc1": e1,
                "esrc2": e2,
                "pdloc": bf(pdl[c]),
            }
        )
    return cfg, c2_const, in_maps


_CACHE = {}
LAST_RESULT = None


def kernel(**inputs) -> np.ndarray:
    from concourse.bass_utils import run_bass_kernel_spmd

    global LAST_RESULT
    x = np.asarray(inputs["x"])
    nb, ncn, d = x.shape
    cfg, c2_const, in_maps = prepare(**{k: inputs[k] for k in (
        "x", "seq", "edges", "W1", "att_src1", "att_dst1", "b1",
        "W2", "att_src2", "att_dst2", "b2")})

    key = (cfg.N, cfg.D, cfg.H1, cfg.C1, cfg.D2, cfg.T, KP, cfg.NSLICE,
           round(c2_const, 10))
    if key not in _CACHE:
        _CACHE.clear()
        _CACHE[key] = build_program(cfg, c2_const)
    nc = _CACHE[key]

    res = run_bass_kernel_spmd(nc, in_maps, core_ids=list(range(NCORES)), trace=False)
    LAST_RESULT = res
    shards = [res.results[c]["out"] for c in range(NCORES)]
    full = np.concatenate(shards, axis=0)
    return full.reshape(nb, ncn, d).astype(np.float32)
